# revision 4
# baseline (speedup 1.0000x reference)
"""HardAndLayer on 8 Trainium2 NeuronCores — raw-bass v2.

out[l] = AND_d (x[d] OR NOT w[l,d])  ==  no d with (w[l,d] AND NOT x[d])

Wire format: 31-bit packed words (bit 30 held zero, so no fp32 NaN/Inf
pattern). Hand-rolled bass program (no TileContext): manual semaphores,
HWDGE input chunks sized so the DVE starts early and the DMA bus never
starves, and the output leaves via a pre-staged SWDGE scatter-add fired by
trigger_dma right after the last DVE op (skips HWDGE 625ns + DGE 650ns on
the tail). res DRAM is pre-zeroed by a small DMA; the LOGICAL_OR fold
yields exact 0.0/1.0 flags, so the fp32 scatter-ADD is bit-exact.
Host: out[neuron] = all per-piece flags == 0.
"""

import numpy as np

L = 8192
D = 8192
NCORES = 8
LSH = L // NCORES  # 1024 neuron rows per core
# 31 bits per word with bit 30 held zero: no word can form a NaN/Inf
# pattern. Dense 32-bit packing was tried and FAILS on HW — an all-ones
# notx word (x=0) is a NaN pattern and the DVE read datapath mangles it
# (1780/8192 mismatches on the adversarial x=all-zeros case).
PAYLOAD = 31
WPK = -(-D // PAYLOAD)  # packed words per neuron row
NB = LSH // 128  # 8 rows per partition

ROWS_W = NB * WPK  # 2120 row words per partition
TOT_W = (NB + 1) * WPK  # + notx
RES_STRIDE = 64  # res row stride in f32 (256B scatter-add constraint)

# ---- schedule config (tuned against TimelineSim) ----
CFG = {
    # chunk boundaries (words) over [notx | r0..r7] (9*WPK total)
    "bounds": (0, 725, 1150, 1590, 1855, 2230, 2385),
    # alternate SP/Act DMA issue: two sequencers feed the global HWDGE unit,
    # so descriptor-gen slots pace at 625ns instead of one engine's 650ns
    "alt_engines": True,
    # number of leading chunks delivered via Pool SWDGE prepare+trigger
    # (gather with identity idxs) instead of HWDGE dma_start. A trig chunk's
    # width must be a multiple of 64 words (256B gather elem constraint).
    "n_trig": 0,
    "zero_mode": "sp_last",  # "sp_last" | "pool"
    "min_piece": 8,  # merge op pieces smaller than this into neighbor
    # "scatter": pre-staged SWDGE scatter-add fired by trigger_dma (fast tail)
    # "plain": SP HWDGE dma_start of acc -> res after DVE done (safe fallback)
    "out_mode": "scatter",
}

_BITPOS = list(range(30)) + [31]
DPAD = WPK * PAYLOAD

_compiled = None
_custom_op = None


def _register_custom_op():
    global _custom_op
    if _custom_op is not None:
        return _custom_op
    from concourse import dve_ops
    from concourse.dve_spec import Spec, Src0, Src1, Zero, Bin, lower
    from concourse.dve_uop import AluOp, DveOpSpec

    name = "AND_ANY_ANT"
    for o in dve_ops.OPS:
        if o.name == name:
            _custom_op = o
            return o

    def _ref(in0, in1, c0, c1, c2):
        a = in0.view(np.uint32) & in1.view(np.uint32)
        acc = (
            (a.reshape(a.shape[0], -1) != 0)
            .any(axis=-1, keepdims=True)
            .astype(np.float32)
        )
        return a.view(np.float32), acc

    spec = Spec(
        body=Bin(AluOp.BITWISE_AND, Src0, Src1),
        accum=AluOp.LOGICAL_OR,
        accum_init=Zero,
        reference=_ref,
    )
    shas = {}
    for ver in ("v3", "v4"):
        try:
            uops = lower(spec, ver=ver)
            shas[ver] = DveOpSpec(name=name, uops=uops, rd1_en=True).sha(ver)
        except Exception:
            pass
    op = dve_ops.DveOp(name, spec, subdim=False, uops_sha=shas)
    dve_ops.OPS.append(op)
    dve_ops._SUB_OPCODE_FOR_NAME[name] = (
        dve_ops._CUSTOM_DVE_ROW_BASE + len(dve_ops.OPS) - 1
    )
    dve_ops.CUSTOM_DVE_SPECS[name] = spec
    _custom_op = op
    return op


def _plan_ilv(cfg):
    """Interleaved layout: [ilv(nx,r0) 2*WPK | r1 .. r7]. Chunk 0 can be
    half-sized (notx+r0 words arrive as pairs), starting the DVE earlier;
    rows 1-7 read notx through a stride-2 AP over the interleave region.

    ops: (row, w0, w1, wait_chunks) with row-relative word ranges.
    """
    bounds = list(cfg["bounds"])
    assert bounds[0] == 0 and bounds[-1] == TOT_W
    ILV = 2 * WPK
    for b in bounds:
        if b < ILV:
            assert b % 2 == 0, "bounds inside the interleave region must be even"
    chunks = [(bounds[i], bounds[i + 1]) for i in range(len(bounds) - 1)]

    def chunk_of(col):
        for ci, (a, b) in enumerate(chunks):
            if a <= col < b:
                return ci
        raise AssertionError(col)

    ilv_chunk = chunk_of(ILV - 1)  # last chunk holding interleave words
    min_piece = cfg.get("min_piece", 8)

    def pieces_of(lo, hi, cutpts):
        cuts = [lo] + [c for c in cutpts if lo < c < hi] + [hi]
        out = []
        for i in range(len(cuts) - 1):
            a, b = cuts[i], cuts[i + 1]
            if out and (b - a) < min_piece:
                pa, _ = out.pop()
                out.append((pa, b))
            else:
                out.append((a, b))
        return out

    ops = []
    # r0: pair-space cuts at even bounds inside [0, ILV)
    cutp = [b // 2 for b in bounds if 0 < b < ILV]
    for a, b in pieces_of(0, WPK, cutp):
        ops.append((0, a, b, (chunk_of(2 * b - 1),)))
    # rows 1-7
    for r in range(1, NB):
        base = ILV + (r - 1) * WPK
        cutw = [b - base for b in bounds if base < b < base + WPK]
        for a, b in pieces_of(0, WPK, cutw):
            ops.append((r, a, b, (chunk_of(base + b - 1), ilv_chunk)))
    return chunks, ops


def _plan(cfg):
    """Derive (chunks, ops) from cfg.

    chunks: list of (w0, w1) word ranges over [notx | rows] layout.
    ops: list of (row, w0, w1, chunk_idx) — row-relative word range, the op
    is issued after `chunk_idx`'s DMA lands (notx chunk 0 is waited first).
    """
    bounds = list(cfg["bounds"])
    assert bounds[0] == 0 and bounds[-1] == TOT_W
    assert bounds[1] >= WPK, "chunk 0 must cover notx"
    chunks = [(bounds[i], bounds[i + 1]) for i in range(len(bounds) - 1)]

    def chunk_of(col):
        for ci, (a, b) in enumerate(chunks):
            if a <= col < b:
                return ci
        raise AssertionError(col)

    min_piece = cfg.get("min_piece", 8)
    ops = []
    for r in range(NB):
        lo, hi = (1 + r) * WPK, (2 + r) * WPK
        cuts = [lo] + [b for b in bounds if lo < b < hi] + [hi]
        # merge pieces smaller than min_piece into the previous piece
        pieces = []
        for i in range(len(cuts) - 1):
            a, b = cuts[i], cuts[i + 1]
            if pieces and (b - a) < min_piece:
                pa, _ = pieces.pop()
                pieces.append((pa, b))
            else:
                pieces.append((a, b))
        for a, b in pieces:
            ops.append((r, a - lo, b - lo, chunk_of(b - 1)))
    return chunks, ops


def _build(cfg=None):
    import concourse.bacc as bacc
    import concourse.mybir as mybir
    from concourse.library_config import mlp
    from contextlib import ExitStack

    if cfg is None:
        cfg = CFG
    op = _register_custom_op()
    ilv = bool(cfg.get("interleave"))
    chunks, ops = (_plan_ilv if ilv else _plan)(cfg)
    n_trig = cfg.get("n_trig", 0)
    for a, b in chunks[:n_trig]:
        assert (b - a) % 64 == 0, "trig chunk width must be 256B-aligned"
    nacc = len(ops)
    assert nacc <= RES_STRIDE
    zcol = TOT_W  # zero-source words live after the weights
    tot = -(-(TOT_W + nacc) // 64) * 64  # row stride %64 (gather elem_step)

    nc = bacc.Bacc(
        "TRN2",
        target_bir_lowering=False,
        debug=False,
        enable_asserts=False,
        num_devices=NCORES,
    )
    wx = nc.dram_tensor("wx", [128, tot], mybir.dt.float32, kind="ExternalInput")
    res = nc.dram_tensor(
        "res", [128, RES_STRIDE], mybir.dt.float32, kind="ExternalOutput"
    )
    # NOTE: no other ExternalInput may be declared — bass2jax passes exactly
    # the tensors in in_maps; a declared-but-unfed input fails the execute.

    with (
        ExitStack() as stack,
        nc.sbuf_tensor("wsb", [128, TOT_W], mybir.dt.float32) as wsb,
        nc.sbuf_tensor("idxs_sb", [128, 8], mybir.dt.int16) as idxs_sb,
        nc.sbuf_tensor("acc", [128, 1, RES_STRIDE], mybir.dt.float32) as acc,
        nc.sbuf_tensor("m", [128, len(ops), WPK], mybir.dt.float32) as m,
        nc.semaphore("zs") as zs,
        nc.semaphore("ds") as ds,
        nc.semaphore("ps") as ps,
        nc.semaphore("fs") as fs,
    ):
        csems = [
            stack.enter_context(nc.semaphore(f"c{i}")) for i in range(len(chunks))
        ]
        gps = stack.enter_context(nc.semaphore("gps"))

        # --- SP/Act: input chunks via HWDGE, then res pre-zero. Two
        # sequencers feed the (global) HWDGE unit so descriptor-gen slots
        # pace at 625ns instead of one engine's 650ns DMA_SEQ_TIME. Note
        # DGE_DMA_DELAY differs per engine (SP 650 vs Act 784), so the
        # per-DMA engine assignment is schedule-searched, not just
        # alternated. "engines": 0=sync, 1=scalar per DMA (zero DMA last).
        alt = cfg.get("alt_engines", False)
        engs = cfg.get("engines")
        if engs is None:
            n_dma = len(chunks) - n_trig + 1
            engs = tuple(i % 2 if alt else 0 for i in range(n_dma))
        hw_i = 0
        for ci, (a, b) in enumerate(chunks):
            if ci < n_trig:
                continue
            eng = nc.scalar if engs[hw_i] else nc.sync
            eng.dma_start(wsb[:, a:b], wx[:, a:b]).then_inc(csems[ci], 16)
            hw_i += 1
        if cfg["zero_mode"] == "sp_last" and cfg.get("out_mode", "scatter") == "scatter":
            eng = nc.scalar if engs[hw_i] else nc.sync
            eng.dma_start(res[:, 0:nacc], wx[:, zcol : zcol + nacc]).then_inc(
                zs, 16
            )

        # --- DVE: fused AND+any per op piece ---
        waited = set()
        nx_ap = wsb[:, 0:WPK]
        inst = None
        for j, (r, w0, w1, ci) in enumerate(ops):
            need = ci if isinstance(ci, tuple) else (0, ci)
            for c in sorted(set(need)):
                if c not in waited:
                    nc.vector.wait_ge(csems[c], 16)
                    waited.add(c)
            if ilv:
                if r == 0:
                    in0 = wsb[:, 2 * w0 + 1 : 2 * w1 : 2]
                    in1 = wsb[:, 2 * w0 : 2 * w1 : 2]
                else:
                    base = (1 + r) * WPK
                    in0 = wsb[:, base + w0 : base + w1]
                    in1 = wsb[:, 2 * w0 : 2 * w1 : 2]
            else:
                in0 = wsb[:, (1 + r) * WPK + w0 : (1 + r) * WPK + w1]
                in1 = nx_ap[:, w0:w1]
            inst = nc.vector._custom_dve(
                op,
                out=m[:, j, 0 : w1 - w0],
                in0=in0,
                in1=in1,
                accum_out=acc[:, 0, j : j + 1],
            )
        inst.then_inc(ds, 1)

        if cfg.get("out_mode", "scatter") == "plain":
            # safe fallback: plain HWDGE out-DMA on SP after DVE done
            nc.sync.wait_ge(ds, 1)
            nc.sync.dma_start(res[:, 0:nacc], acc[:, 0, 0:nacc]).then_inc(fs, 16)
            nc.sync.wait_ge(fs, 16)
            nc.compile()
            nc._ant_plan = (chunks, ops, tot, ilv)
            return nc

        # --- Pool: identity idxs via iota (boot lib = standard), early input
        # chunks + scatter prep staged in the SWDGE ring; input triggers fire
        # immediately, the scatter trigger right after the last DVE op ---
        isem = stack.enter_context(nc.semaphore("isem"))
        nc.gpsimd.memset(idxs_sb[:, :], 0).then_inc(isem, 1)
        nc.gpsimd.wait_ge(isem, 1)
        nc.gpsimd.iota(
            idxs_sb[0:16, :], [[16, 8]], base=0, channel_multiplier=1
        ).then_inc(isem, 1)
        nc.gpsimd.load_library(mlp)
        nc.gpsimd.wait_ge(isem, 2)
        for ci in range(n_trig):
            a, b = chunks[ci]
            nc.gpsimd.dma_gather(
                wsb[:, a:b].unsqueeze(1),
                wx[:, a:b],
                idxs_sb[:, :],
                128,
                128,
                b - a,
                elem_step=tot,
                prepare_only=True,
                sem=csems[ci],
            ).then_inc(gps, 1)
            nc.gpsimd.wait_ge(gps, ci + 1)
            nc.gpsimd.trigger_dma(1)
        if cfg["zero_mode"] == "pool":
            nc.gpsimd.dma_start(
                res[:, 0:nacc], wx[:, zcol : zcol + nacc]
            ).then_inc(zs, 16)
        nc.gpsimd.dma_scatter_add(
            res[:, 0:nacc],
            acc[:, :, 0:nacc],
            idxs_sb[:, :],
            128,
            128,
            nacc,
            elem_step=RES_STRIDE,
            prepare_only=True,
            sem=fs,
        ).then_inc(ps, 1)
        # ds last: ps/zs resolve long before the final DVE op, so the SEQ
        # sits parked on ds and the trigger dispatches right after it fires
        nc.gpsimd.wait_ge(ps, 1)
        nc.gpsimd.wait_ge(zs, 16)
        nc.gpsimd.wait_ge(ds, 1)
        nc.gpsimd.trigger_dma(1)
        nc.gpsimd.wait_ge(fs, 16)

    nc.compile()
    nc._ant_plan = (chunks, ops, tot, ilv)  # host-side gather map
    return nc


def _pack31(bits):
    lead = bits.shape[:-1]
    if PAYLOAD == 32:
        words = np.packbits(bits, axis=-1, bitorder="little")
        return words.view(np.uint32).view(np.float32)
    b32 = np.zeros(lead + (WPK, 32), dtype=np.uint8)
    pad = np.zeros(lead + (DPAD,), dtype=np.uint8)
    pad[..., :D] = bits
    pad = pad.reshape(lead + (WPK, PAYLOAD))
    b32[..., :30] = pad[..., :30]
    b32[..., 31] = pad[..., 30]
    words = np.packbits(b32.reshape(lead + (WPK * 32,)), axis=-1, bitorder="little")
    return words.view(np.uint32).view(np.float32)


def _pack_inputs(x, bit_weights, tot, ilv=False):
    x = np.asarray(x).astype(np.uint8)
    bw = np.ascontiguousarray(np.asarray(bit_weights).astype(np.uint8))
    notx = (1 - x).astype(np.uint8)
    nxp = _pack31(notx)  # [WPK]
    wp = _pack31(bw)  # [L, WPK]
    in_maps = []
    for i in range(NCORES):
        shard = wp[i * LSH : (i + 1) * LSH].reshape(128, NB, WPK)
        wxa = np.zeros((128, tot), dtype=np.float32)
        if ilv:
            wxa[:, 0 : 2 * WPK : 2] = nxp
            wxa[:, 1 : 2 * WPK : 2] = shard[:, 0, :]
            wxa[:, 2 * WPK : TOT_W] = shard[:, 1:, :].reshape(128, ROWS_W - WPK)
        else:
            wxa[:, 0:WPK] = nxp
            wxa[:, WPK:TOT_W] = shard.reshape(128, ROWS_W)
        in_maps.append({"wx": wxa})
    return in_maps


def _gather(results, ops):
    outs = []
    for i in range(NCORES):
        res = results[i]["res"]  # [128, RES_STRIDE] fp32 flags
        viol = np.zeros((128, NB), dtype=bool)
        for j, (r, _, _, _) in enumerate(ops):
            viol[:, r] |= res[:, j] != 0.0
        outs.append(~viol.reshape(-1))
    return np.concatenate(outs).astype(np.bool_)


def _get_compiled():
    global _compiled
    if _compiled is None:
        _compiled = _build()
    return _compiled


def kernel(x, bit_weights):
    from concourse import bass_utils

    nc = _get_compiled()
    chunks, ops, tot, ilv = nc._ant_plan
    in_maps = _pack_inputs(x, bit_weights, tot, ilv)
    r = bass_utils.run_bass_kernel_spmd(nc, in_maps, core_ids=list(range(NCORES)))
    return _gather(r.results, ops)


# revision 5
# speedup vs baseline: 1.0019x; 1.0019x over previous
"""HardAndLayer on 8 Trainium2 NeuronCores — raw-bass v2.

out[l] = AND_d (x[d] OR NOT w[l,d])  ==  no d with (w[l,d] AND NOT x[d])

Wire format: 31-bit packed words (bit 30 held zero, so no fp32 NaN/Inf
pattern). Hand-rolled bass program (no TileContext): manual semaphores,
HWDGE input chunks sized so the DVE starts early and the DMA bus never
starves, and the output leaves via a pre-staged SWDGE scatter-add fired by
trigger_dma right after the last DVE op (skips HWDGE 625ns + DGE 650ns on
the tail). res DRAM is pre-zeroed by a small DMA; the LOGICAL_OR fold
yields exact 0.0/1.0 flags, so the fp32 scatter-ADD is bit-exact.
Host: out[neuron] = all per-piece flags == 0.
"""

import numpy as np

L = 8192
D = 8192
NCORES = 8
LSH = L // NCORES  # 1024 neuron rows per core
# 31 bits per word with bit 30 held zero: no word can form a NaN/Inf
# pattern. Dense 32-bit packing was tried and FAILS on HW — an all-ones
# notx word (x=0) is a NaN pattern and the DVE read datapath mangles it
# (1780/8192 mismatches on the adversarial x=all-zeros case).
PAYLOAD = 31
WPK = -(-D // PAYLOAD)  # packed words per neuron row
NB = LSH // 128  # 8 rows per partition

ROWS_W = NB * WPK  # 2120 row words per partition
TOT_W = (NB + 1) * WPK  # + notx
RES_STRIDE = 64  # res row stride in f32 (256B scatter-add constraint)

# ---- schedule config (tuned against TimelineSim) ----
CFG = {
    # interleaved layout [ilv(nx,r0) | r1..r7]: chunk 0 carries notx+r0 as
    # pairs, so the DVE starts ~140ns earlier; rows 1-7 read notx through a
    # stride-2 AP. Bounds are words over that layout (9*WPK total).
    "interleave": True,
    "bounds": (0, 642, 1060, 1468, 1855, 2162, 2385),
    # alternate SP/Act DMA issue: two sequencers feed the global HWDGE unit,
    # so descriptor-gen slots pace at 625ns instead of one engine's 650ns
    "alt_engines": True,
    # number of leading chunks delivered via Pool SWDGE prepare+trigger
    # (gather with identity idxs) instead of HWDGE dma_start. A trig chunk's
    # width must be a multiple of 64 words (256B gather elem constraint).
    "n_trig": 0,
    "zero_mode": "sp_last",  # "sp_last" | "pool"
    "min_piece": 8,  # merge op pieces smaller than this into neighbor
    # "scatter": pre-staged SWDGE scatter-add fired by trigger_dma (fast tail)
    # "plain": SP HWDGE dma_start of acc -> res after DVE done (safe fallback)
    "out_mode": "scatter",
}

_BITPOS = list(range(30)) + [31]
DPAD = WPK * PAYLOAD

_compiled = None
_custom_op = None


def _register_custom_op():
    global _custom_op
    if _custom_op is not None:
        return _custom_op
    from concourse import dve_ops
    from concourse.dve_spec import Spec, Src0, Src1, Zero, Bin, lower
    from concourse.dve_uop import AluOp, DveOpSpec

    name = "AND_ANY_ANT"
    for o in dve_ops.OPS:
        if o.name == name:
            _custom_op = o
            return o

    def _ref(in0, in1, c0, c1, c2):
        a = in0.view(np.uint32) & in1.view(np.uint32)
        acc = (
            (a.reshape(a.shape[0], -1) != 0)
            .any(axis=-1, keepdims=True)
            .astype(np.float32)
        )
        return a.view(np.float32), acc

    spec = Spec(
        body=Bin(AluOp.BITWISE_AND, Src0, Src1),
        accum=AluOp.LOGICAL_OR,
        accum_init=Zero,
        reference=_ref,
    )
    shas = {}
    for ver in ("v3", "v4"):
        try:
            uops = lower(spec, ver=ver)
            shas[ver] = DveOpSpec(name=name, uops=uops, rd1_en=True).sha(ver)
        except Exception:
            pass
    op = dve_ops.DveOp(name, spec, subdim=False, uops_sha=shas)
    dve_ops.OPS.append(op)
    dve_ops._SUB_OPCODE_FOR_NAME[name] = (
        dve_ops._CUSTOM_DVE_ROW_BASE + len(dve_ops.OPS) - 1
    )
    dve_ops.CUSTOM_DVE_SPECS[name] = spec
    _custom_op = op
    return op


def _plan_ilv(cfg):
    """Interleaved layout: [ilv(nx,r0) 2*WPK | r1 .. r7]. Chunk 0 can be
    half-sized (notx+r0 words arrive as pairs), starting the DVE earlier;
    rows 1-7 read notx through a stride-2 AP over the interleave region.

    ops: (row, w0, w1, wait_chunks) with row-relative word ranges.
    """
    bounds = list(cfg["bounds"])
    assert bounds[0] == 0 and bounds[-1] == TOT_W
    ILV = 2 * WPK
    for b in bounds:
        if b < ILV:
            assert b % 2 == 0, "bounds inside the interleave region must be even"
    chunks = [(bounds[i], bounds[i + 1]) for i in range(len(bounds) - 1)]

    def chunk_of(col):
        for ci, (a, b) in enumerate(chunks):
            if a <= col < b:
                return ci
        raise AssertionError(col)

    ilv_chunk = chunk_of(ILV - 1)  # last chunk holding interleave words
    min_piece = cfg.get("min_piece", 8)

    def pieces_of(lo, hi, cutpts):
        cuts = [lo] + [c for c in cutpts if lo < c < hi] + [hi]
        out = []
        for i in range(len(cuts) - 1):
            a, b = cuts[i], cuts[i + 1]
            if out and (b - a) < min_piece:
                pa, _ = out.pop()
                out.append((pa, b))
            else:
                out.append((a, b))
        return out

    ops = []
    # r0: pair-space cuts at even bounds inside [0, ILV)
    cutp = [b // 2 for b in bounds if 0 < b < ILV]
    for a, b in pieces_of(0, WPK, cutp):
        ops.append((0, a, b, (chunk_of(2 * b - 1),)))
    # rows 1-7
    for r in range(1, NB):
        base = ILV + (r - 1) * WPK
        cutw = [b - base for b in bounds if base < b < base + WPK]
        for a, b in pieces_of(0, WPK, cutw):
            ops.append((r, a, b, (chunk_of(base + b - 1), ilv_chunk)))
    return chunks, ops


def _plan(cfg):
    """Derive (chunks, ops) from cfg.

    chunks: list of (w0, w1) word ranges over [notx | rows] layout.
    ops: list of (row, w0, w1, chunk_idx) — row-relative word range, the op
    is issued after `chunk_idx`'s DMA lands (notx chunk 0 is waited first).
    """
    bounds = list(cfg["bounds"])
    assert bounds[0] == 0 and bounds[-1] == TOT_W
    assert bounds[1] >= WPK, "chunk 0 must cover notx"
    chunks = [(bounds[i], bounds[i + 1]) for i in range(len(bounds) - 1)]

    def chunk_of(col):
        for ci, (a, b) in enumerate(chunks):
            if a <= col < b:
                return ci
        raise AssertionError(col)

    min_piece = cfg.get("min_piece", 8)
    ops = []
    for r in range(NB):
        lo, hi = (1 + r) * WPK, (2 + r) * WPK
        cuts = [lo] + [b for b in bounds if lo < b < hi] + [hi]
        # merge pieces smaller than min_piece into the previous piece
        pieces = []
        for i in range(len(cuts) - 1):
            a, b = cuts[i], cuts[i + 1]
            if pieces and (b - a) < min_piece:
                pa, _ = pieces.pop()
                pieces.append((pa, b))
            else:
                pieces.append((a, b))
        for a, b in pieces:
            ops.append((r, a - lo, b - lo, chunk_of(b - 1)))
    return chunks, ops


def _build(cfg=None):
    import concourse.bacc as bacc
    import concourse.mybir as mybir
    from concourse.library_config import mlp
    from contextlib import ExitStack

    if cfg is None:
        cfg = CFG
    op = _register_custom_op()
    ilv = bool(cfg.get("interleave"))
    chunks, ops = (_plan_ilv if ilv else _plan)(cfg)
    n_trig = cfg.get("n_trig", 0)
    for a, b in chunks[:n_trig]:
        assert (b - a) % 64 == 0, "trig chunk width must be 256B-aligned"
    nacc = len(ops)
    assert nacc <= RES_STRIDE
    zcol = TOT_W  # zero-source words live after the weights
    tot = -(-(TOT_W + nacc) // 64) * 64  # row stride %64 (gather elem_step)

    nc = bacc.Bacc(
        "TRN2",
        target_bir_lowering=False,
        debug=False,
        enable_asserts=False,
        num_devices=NCORES,
    )
    wx = nc.dram_tensor("wx", [128, tot], mybir.dt.float32, kind="ExternalInput")
    res = nc.dram_tensor(
        "res", [128, RES_STRIDE], mybir.dt.float32, kind="ExternalOutput"
    )
    # NOTE: no other ExternalInput may be declared — bass2jax passes exactly
    # the tensors in in_maps; a declared-but-unfed input fails the execute.

    with (
        ExitStack() as stack,
        nc.sbuf_tensor("wsb", [128, TOT_W], mybir.dt.float32) as wsb,
        nc.sbuf_tensor("idxs_sb", [128, 8], mybir.dt.int16) as idxs_sb,
        nc.sbuf_tensor("acc", [128, 1, RES_STRIDE], mybir.dt.float32) as acc,
        nc.sbuf_tensor("m", [128, len(ops), WPK], mybir.dt.float32) as m,
        nc.semaphore("zs") as zs,
        nc.semaphore("ds") as ds,
        nc.semaphore("ps") as ps,
        nc.semaphore("fs") as fs,
    ):
        csems = [
            stack.enter_context(nc.semaphore(f"c{i}")) for i in range(len(chunks))
        ]
        gps = stack.enter_context(nc.semaphore("gps"))

        # --- SP/Act: input chunks via HWDGE, then res pre-zero. Two
        # sequencers feed the (global) HWDGE unit so descriptor-gen slots
        # pace at 625ns instead of one engine's 650ns DMA_SEQ_TIME. Note
        # DGE_DMA_DELAY differs per engine (SP 650 vs Act 784), so the
        # per-DMA engine assignment is schedule-searched, not just
        # alternated. "engines": 0=sync, 1=scalar per DMA (zero DMA last).
        alt = cfg.get("alt_engines", False)
        engs = cfg.get("engines")
        if engs is None:
            n_dma = len(chunks) - n_trig + 1
            engs = tuple(i % 2 if alt else 0 for i in range(n_dma))
        hw_i = 0
        for ci, (a, b) in enumerate(chunks):
            if ci < n_trig:
                continue
            eng = nc.scalar if engs[hw_i] else nc.sync
            eng.dma_start(wsb[:, a:b], wx[:, a:b]).then_inc(csems[ci], 16)
            hw_i += 1
        if cfg["zero_mode"] == "sp_last" and cfg.get("out_mode", "scatter") == "scatter":
            eng = nc.scalar if engs[hw_i] else nc.sync
            eng.dma_start(res[:, 0:nacc], wx[:, zcol : zcol + nacc]).then_inc(
                zs, 16
            )

        # --- DVE: fused AND+any per op piece ---
        waited = set()
        nx_ap = wsb[:, 0:WPK]
        inst = None
        for j, (r, w0, w1, ci) in enumerate(ops):
            need = ci if isinstance(ci, tuple) else (0, ci)
            for c in sorted(set(need)):
                if c not in waited:
                    nc.vector.wait_ge(csems[c], 16)
                    waited.add(c)
            if ilv:
                if r == 0:
                    in0 = wsb[:, 2 * w0 + 1 : 2 * w1 : 2]
                    in1 = wsb[:, 2 * w0 : 2 * w1 : 2]
                else:
                    base = (1 + r) * WPK
                    in0 = wsb[:, base + w0 : base + w1]
                    in1 = wsb[:, 2 * w0 : 2 * w1 : 2]
            else:
                in0 = wsb[:, (1 + r) * WPK + w0 : (1 + r) * WPK + w1]
                in1 = nx_ap[:, w0:w1]
            inst = nc.vector._custom_dve(
                op,
                out=m[:, j, 0 : w1 - w0],
                in0=in0,
                in1=in1,
                accum_out=acc[:, 0, j : j + 1],
            )
        inst.then_inc(ds, 1)

        if cfg.get("out_mode", "scatter") == "plain":
            # safe fallback: plain HWDGE out-DMA on SP after DVE done
            nc.sync.wait_ge(ds, 1)
            nc.sync.dma_start(res[:, 0:nacc], acc[:, 0, 0:nacc]).then_inc(fs, 16)
            nc.sync.wait_ge(fs, 16)
            nc.compile()
            nc._ant_plan = (chunks, ops, tot, ilv)
            return nc

        # --- Pool: identity idxs via iota (boot lib = standard), early input
        # chunks + scatter prep staged in the SWDGE ring; input triggers fire
        # immediately, the scatter trigger right after the last DVE op ---
        isem = stack.enter_context(nc.semaphore("isem"))
        nc.gpsimd.memset(idxs_sb[:, :], 0).then_inc(isem, 1)
        nc.gpsimd.wait_ge(isem, 1)
        nc.gpsimd.iota(
            idxs_sb[0:16, :], [[16, 8]], base=0, channel_multiplier=1
        ).then_inc(isem, 1)
        nc.gpsimd.load_library(mlp)
        nc.gpsimd.wait_ge(isem, 2)
        for ci in range(n_trig):
            a, b = chunks[ci]
            nc.gpsimd.dma_gather(
                wsb[:, a:b].unsqueeze(1),
                wx[:, a:b],
                idxs_sb[:, :],
                128,
                128,
                b - a,
                elem_step=tot,
                prepare_only=True,
                sem=csems[ci],
            ).then_inc(gps, 1)
            nc.gpsimd.wait_ge(gps, ci + 1)
            nc.gpsimd.trigger_dma(1)
        if cfg["zero_mode"] == "pool":
            nc.gpsimd.dma_start(
                res[:, 0:nacc], wx[:, zcol : zcol + nacc]
            ).then_inc(zs, 16)
        nc.gpsimd.dma_scatter_add(
            res[:, 0:nacc],
            acc[:, :, 0:nacc],
            idxs_sb[:, :],
            128,
            128,
            nacc,
            elem_step=RES_STRIDE,
            prepare_only=True,
            sem=fs,
        ).then_inc(ps, 1)
        # ds last: ps/zs resolve long before the final DVE op, so the SEQ
        # sits parked on ds and the trigger dispatches right after it fires
        nc.gpsimd.wait_ge(ps, 1)
        nc.gpsimd.wait_ge(zs, 16)
        nc.gpsimd.wait_ge(ds, 1)
        nc.gpsimd.trigger_dma(1)
        nc.gpsimd.wait_ge(fs, 16)

    nc.compile()
    nc._ant_plan = (chunks, ops, tot, ilv)  # host-side gather map
    return nc


def _pack31(bits):
    lead = bits.shape[:-1]
    if PAYLOAD == 32:
        words = np.packbits(bits, axis=-1, bitorder="little")
        return words.view(np.uint32).view(np.float32)
    b32 = np.zeros(lead + (WPK, 32), dtype=np.uint8)
    pad = np.zeros(lead + (DPAD,), dtype=np.uint8)
    pad[..., :D] = bits
    pad = pad.reshape(lead + (WPK, PAYLOAD))
    b32[..., :30] = pad[..., :30]
    b32[..., 31] = pad[..., 30]
    words = np.packbits(b32.reshape(lead + (WPK * 32,)), axis=-1, bitorder="little")
    return words.view(np.uint32).view(np.float32)


def _pack_inputs(x, bit_weights, tot, ilv=False):
    x = np.asarray(x).astype(np.uint8)
    bw = np.ascontiguousarray(np.asarray(bit_weights).astype(np.uint8))
    notx = (1 - x).astype(np.uint8)
    nxp = _pack31(notx)  # [WPK]
    wp = _pack31(bw)  # [L, WPK]
    in_maps = []
    for i in range(NCORES):
        shard = wp[i * LSH : (i + 1) * LSH].reshape(128, NB, WPK)
        wxa = np.zeros((128, tot), dtype=np.float32)
        if ilv:
            wxa[:, 0 : 2 * WPK : 2] = nxp
            wxa[:, 1 : 2 * WPK : 2] = shard[:, 0, :]
            wxa[:, 2 * WPK : TOT_W] = shard[:, 1:, :].reshape(128, ROWS_W - WPK)
        else:
            wxa[:, 0:WPK] = nxp
            wxa[:, WPK:TOT_W] = shard.reshape(128, ROWS_W)
        in_maps.append({"wx": wxa})
    return in_maps


def _gather(results, ops):
    outs = []
    for i in range(NCORES):
        res = results[i]["res"]  # [128, RES_STRIDE] fp32 flags
        viol = np.zeros((128, NB), dtype=bool)
        for j, (r, _, _, _) in enumerate(ops):
            viol[:, r] |= res[:, j] != 0.0
        outs.append(~viol.reshape(-1))
    return np.concatenate(outs).astype(np.bool_)


def _get_compiled():
    global _compiled
    if _compiled is None:
        _compiled = _build()
    return _compiled


def kernel(x, bit_weights):
    from concourse import bass_utils

    nc = _get_compiled()
    chunks, ops, tot, ilv = nc._ant_plan
    in_maps = _pack_inputs(x, bit_weights, tot, ilv)
    r = bass_utils.run_bass_kernel_spmd(nc, in_maps, core_ids=list(range(NCORES)))
    return _gather(r.results, ops)


# revision 6
# speedup vs baseline: 1.0688x; 1.0667x over previous
"""HardAndLayer on 8 Trainium2 NeuronCores — raw-bass v2.

out[l] = AND_d (x[d] OR NOT w[l,d])  ==  no d with (w[l,d] AND NOT x[d])

Wire format: 31-bit packed words (bit 30 held zero, so no fp32 NaN/Inf
pattern). Hand-rolled bass program (no TileContext): manual semaphores,
HWDGE input chunks sized so the DVE starts early and the DMA bus never
starves, and the output leaves via a pre-staged SWDGE scatter-add fired by
trigger_dma right after the last DVE op (skips HWDGE 625ns + DGE 650ns on
the tail). res DRAM is pre-zeroed by a small DMA; the LOGICAL_OR fold
yields exact 0.0/1.0 flags, so the fp32 scatter-ADD is bit-exact.
Host: out[neuron] = all per-piece flags == 0.
"""

import numpy as np

L = 8192
D = 8192
NCORES = 8
LSH = L // NCORES  # 1024 neuron rows per core
# 31 bits per word with bit 30 held zero: no word can form a NaN/Inf
# pattern. Dense 32-bit packing was tried and FAILS on HW — an all-ones
# notx word (x=0) is a NaN pattern and the DVE read datapath mangles it
# (1780/8192 mismatches on the adversarial x=all-zeros case).
PAYLOAD = 31
WPK = -(-D // PAYLOAD)  # packed words per neuron row
NB = LSH // 128  # 8 rows per partition

ROWS_W = NB * WPK  # 2120 row words per partition
TOT_W = (NB + 1) * WPK  # + notx
RES_STRIDE = 64  # res row stride in f32 (256B scatter-add constraint)

# ---- schedule config (tuned against TimelineSim) ----
CFG = {
    # interleaved layout [ilv(nx,r0) | r1..r7]: chunk 0 carries notx+r0 as
    # pairs, so the DVE starts ~140ns earlier; rows 1-7 read notx through a
    # stride-2 AP. Bounds are words over that layout (9*WPK total).
    "interleave": True,
    "bounds": (0, 694, 1108, 1548, 1891, 2282, 2385),
    # alternate SP/Act DMA issue: two sequencers feed the global HWDGE unit,
    # so descriptor-gen slots pace at 625ns instead of one engine's 650ns
    "alt_engines": True,
    # number of leading chunks delivered via Pool SWDGE prepare+trigger
    # (gather with identity idxs) instead of HWDGE dma_start. A trig chunk's
    # width must be a multiple of 64 words (256B gather elem constraint).
    "n_trig": 0,
    "zero_mode": "sp_last",  # "sp_last" | "pool"
    "min_piece": 8,  # merge op pieces smaller than this into neighbor
    # "scatter": pre-staged SWDGE scatter-add fired by trigger_dma (fast tail)
    # "plain": SP HWDGE dma_start of acc -> res after DVE done (safe fallback)
    "out_mode": "scatter",
    # hoist the first SP/Act input DMAs ahead of the framework entry barrier
    # in their engine streams: the first transfer has no cross-engine
    # dependency, so it can overlap the ~616ns preamble (first byte ~1300
    # instead of ~1916)
    "hoist": True,
}

_BITPOS = list(range(30)) + [31]
DPAD = WPK * PAYLOAD

_compiled = None
_custom_op = None


def _register_custom_op():
    global _custom_op
    if _custom_op is not None:
        return _custom_op
    from concourse import dve_ops
    from concourse.dve_spec import Spec, Src0, Src1, Zero, Bin, lower
    from concourse.dve_uop import AluOp, DveOpSpec

    name = "AND_ANY_ANT"
    for o in dve_ops.OPS:
        if o.name == name:
            _custom_op = o
            return o

    def _ref(in0, in1, c0, c1, c2):
        a = in0.view(np.uint32) & in1.view(np.uint32)
        acc = (
            (a.reshape(a.shape[0], -1) != 0)
            .any(axis=-1, keepdims=True)
            .astype(np.float32)
        )
        return a.view(np.float32), acc

    spec = Spec(
        body=Bin(AluOp.BITWISE_AND, Src0, Src1),
        accum=AluOp.LOGICAL_OR,
        accum_init=Zero,
        reference=_ref,
    )
    shas = {}
    for ver in ("v3", "v4"):
        try:
            uops = lower(spec, ver=ver)
            shas[ver] = DveOpSpec(name=name, uops=uops, rd1_en=True).sha(ver)
        except Exception:
            pass
    op = dve_ops.DveOp(name, spec, subdim=False, uops_sha=shas)
    dve_ops.OPS.append(op)
    dve_ops._SUB_OPCODE_FOR_NAME[name] = (
        dve_ops._CUSTOM_DVE_ROW_BASE + len(dve_ops.OPS) - 1
    )
    dve_ops.CUSTOM_DVE_SPECS[name] = spec
    _custom_op = op
    return op


def _plan_ilv(cfg):
    """Interleaved layout: [ilv(nx,r0) 2*WPK | r1 .. r7]. Chunk 0 can be
    half-sized (notx+r0 words arrive as pairs), starting the DVE earlier;
    rows 1-7 read notx through a stride-2 AP over the interleave region.

    ops: (row, w0, w1, wait_chunks) with row-relative word ranges.
    """
    bounds = list(cfg["bounds"])
    assert bounds[0] == 0 and bounds[-1] == TOT_W
    ILV = 2 * WPK
    for b in bounds:
        if b < ILV:
            assert b % 2 == 0, "bounds inside the interleave region must be even"
    chunks = [(bounds[i], bounds[i + 1]) for i in range(len(bounds) - 1)]

    def chunk_of(col):
        for ci, (a, b) in enumerate(chunks):
            if a <= col < b:
                return ci
        raise AssertionError(col)

    ilv_chunk = chunk_of(ILV - 1)  # last chunk holding interleave words
    min_piece = cfg.get("min_piece", 8)

    def pieces_of(lo, hi, cutpts):
        cuts = [lo] + [c for c in cutpts if lo < c < hi] + [hi]
        out = []
        for i in range(len(cuts) - 1):
            a, b = cuts[i], cuts[i + 1]
            if out and (b - a) < min_piece:
                pa, _ = out.pop()
                out.append((pa, b))
            else:
                out.append((a, b))
        return out

    ops = []
    # r0: pair-space cuts at even bounds inside [0, ILV)
    cutp = [b // 2 for b in bounds if 0 < b < ILV]
    for a, b in pieces_of(0, WPK, cutp):
        ops.append((0, a, b, (chunk_of(2 * b - 1),)))
    # rows 1-7
    for r in range(1, NB):
        base = ILV + (r - 1) * WPK
        cutw = [b - base for b in bounds if base < b < base + WPK]
        for a, b in pieces_of(0, WPK, cutw):
            ops.append((r, a, b, (chunk_of(base + b - 1), ilv_chunk)))
    return chunks, ops


def _plan(cfg):
    """Derive (chunks, ops) from cfg.

    chunks: list of (w0, w1) word ranges over [notx | rows] layout.
    ops: list of (row, w0, w1, chunk_idx) — row-relative word range, the op
    is issued after `chunk_idx`'s DMA lands (notx chunk 0 is waited first).
    """
    bounds = list(cfg["bounds"])
    assert bounds[0] == 0 and bounds[-1] == TOT_W
    assert bounds[1] >= WPK, "chunk 0 must cover notx"
    chunks = [(bounds[i], bounds[i + 1]) for i in range(len(bounds) - 1)]

    def chunk_of(col):
        for ci, (a, b) in enumerate(chunks):
            if a <= col < b:
                return ci
        raise AssertionError(col)

    min_piece = cfg.get("min_piece", 8)
    ops = []
    for r in range(NB):
        lo, hi = (1 + r) * WPK, (2 + r) * WPK
        cuts = [lo] + [b for b in bounds if lo < b < hi] + [hi]
        # merge pieces smaller than min_piece into the previous piece
        pieces = []
        for i in range(len(cuts) - 1):
            a, b = cuts[i], cuts[i + 1]
            if pieces and (b - a) < min_piece:
                pa, _ = pieces.pop()
                pieces.append((pa, b))
            else:
                pieces.append((a, b))
        for a, b in pieces:
            ops.append((r, a - lo, b - lo, chunk_of(b - 1)))
    return chunks, ops


def _build(cfg=None):
    import concourse.bacc as bacc
    import concourse.mybir as mybir
    from concourse.library_config import mlp
    from contextlib import ExitStack

    if cfg is None:
        cfg = CFG
    op = _register_custom_op()
    ilv = bool(cfg.get("interleave"))
    chunks, ops = (_plan_ilv if ilv else _plan)(cfg)
    n_trig = cfg.get("n_trig", 0)
    for a, b in chunks[:n_trig]:
        assert (b - a) % 64 == 0, "trig chunk width must be 256B-aligned"
    nacc = len(ops)
    assert nacc <= RES_STRIDE
    zcol = TOT_W  # zero-source words live after the weights
    tot = -(-(TOT_W + nacc) // 64) * 64  # row stride %64 (gather elem_step)

    nc = bacc.Bacc(
        "TRN2",
        target_bir_lowering=False,
        debug=False,
        enable_asserts=False,
        num_devices=NCORES,
    )
    wx = nc.dram_tensor("wx", [128, tot], mybir.dt.float32, kind="ExternalInput")
    res = nc.dram_tensor(
        "res", [128, RES_STRIDE], mybir.dt.float32, kind="ExternalOutput"
    )
    # NOTE: no other ExternalInput may be declared — bass2jax passes exactly
    # the tensors in in_maps; a declared-but-unfed input fails the execute.

    with (
        ExitStack() as stack,
        nc.sbuf_tensor("wsb", [128, TOT_W], mybir.dt.float32) as wsb,
        nc.sbuf_tensor("idxs_sb", [128, 8], mybir.dt.int16) as idxs_sb,
        nc.sbuf_tensor("acc", [128, 1, RES_STRIDE], mybir.dt.float32) as acc,
        nc.sbuf_tensor("m", [128, len(ops), WPK], mybir.dt.float32) as m,
        nc.semaphore("zs") as zs,
        nc.semaphore("ds") as ds,
        nc.semaphore("ps") as ps,
        nc.semaphore("fs") as fs,
    ):
        csems = [
            stack.enter_context(nc.semaphore(f"c{i}")) for i in range(len(chunks))
        ]
        gps = stack.enter_context(nc.semaphore("gps"))

        # --- SP/Act: input chunks via HWDGE, then res pre-zero. Two
        # sequencers feed the (global) HWDGE unit so descriptor-gen slots
        # pace at 625ns instead of one engine's 650ns DMA_SEQ_TIME. Note
        # DGE_DMA_DELAY differs per engine (SP 650 vs Act 784), so the
        # per-DMA engine assignment is schedule-searched, not just
        # alternated. "engines": 0=sync, 1=scalar per DMA (zero DMA last).
        alt = cfg.get("alt_engines", False)
        engs = cfg.get("engines")
        if engs is None:
            n_dma = len(chunks) - n_trig + 1
            engs = tuple(i % 2 if alt else 0 for i in range(n_dma))
        hw_i = 0
        for ci, (a, b) in enumerate(chunks):
            if ci < n_trig:
                continue
            eng = nc.scalar if engs[hw_i] else nc.sync
            eng.dma_start(wsb[:, a:b], wx[:, a:b]).then_inc(csems[ci], 16)
            hw_i += 1
        if cfg["zero_mode"] == "sp_last" and cfg.get("out_mode", "scatter") == "scatter":
            eng = nc.scalar if engs[hw_i] else nc.sync
            eng.dma_start(res[:, 0:nacc], wx[:, zcol : zcol + nacc]).then_inc(
                zs, 16
            )

        # --- DVE: fused AND+any per op piece ---
        waited = set()
        nx_ap = wsb[:, 0:WPK]
        inst = None
        for j, (r, w0, w1, ci) in enumerate(ops):
            need = ci if isinstance(ci, tuple) else (0, ci)
            for c in sorted(set(need)):
                if c not in waited:
                    nc.vector.wait_ge(csems[c], 16)
                    waited.add(c)
            if ilv:
                if r == 0:
                    in0 = wsb[:, 2 * w0 + 1 : 2 * w1 : 2]
                    in1 = wsb[:, 2 * w0 : 2 * w1 : 2]
                else:
                    base = (1 + r) * WPK
                    in0 = wsb[:, base + w0 : base + w1]
                    in1 = wsb[:, 2 * w0 : 2 * w1 : 2]
            else:
                in0 = wsb[:, (1 + r) * WPK + w0 : (1 + r) * WPK + w1]
                in1 = nx_ap[:, w0:w1]
            inst = nc.vector._custom_dve(
                op,
                out=m[:, j, 0 : w1 - w0],
                in0=in0,
                in1=in1,
                accum_out=acc[:, 0, j : j + 1],
            )
        inst.then_inc(ds, 1)

        if cfg.get("out_mode", "scatter") == "plain":
            # safe fallback: plain HWDGE out-DMA on SP after DVE done
            nc.sync.wait_ge(ds, 1)
            nc.sync.dma_start(res[:, 0:nacc], acc[:, 0, 0:nacc]).then_inc(fs, 16)
            nc.sync.wait_ge(fs, 16)
            nc.compile()
            nc._ant_plan = (chunks, ops, tot, ilv)
            return nc

        # --- Pool: identity idxs via iota (boot lib = standard), early input
        # chunks + scatter prep staged in the SWDGE ring; input triggers fire
        # immediately, the scatter trigger right after the last DVE op ---
        isem = stack.enter_context(nc.semaphore("isem"))
        nc.gpsimd.memset(idxs_sb[:, :], 0).then_inc(isem, 1)
        nc.gpsimd.wait_ge(isem, 1)
        nc.gpsimd.iota(
            idxs_sb[0:16, :], [[16, 8]], base=0, channel_multiplier=1
        ).then_inc(isem, 1)
        nc.gpsimd.load_library(mlp)
        nc.gpsimd.wait_ge(isem, 2)
        for ci in range(n_trig):
            a, b = chunks[ci]
            nc.gpsimd.dma_gather(
                wsb[:, a:b].unsqueeze(1),
                wx[:, a:b],
                idxs_sb[:, :],
                128,
                128,
                b - a,
                elem_step=tot,
                prepare_only=True,
                sem=csems[ci],
            ).then_inc(gps, 1)
            nc.gpsimd.wait_ge(gps, ci + 1)
            nc.gpsimd.trigger_dma(1)
        if cfg["zero_mode"] == "pool":
            nc.gpsimd.dma_start(
                res[:, 0:nacc], wx[:, zcol : zcol + nacc]
            ).then_inc(zs, 16)
        nc.gpsimd.dma_scatter_add(
            res[:, 0:nacc],
            acc[:, :, 0:nacc],
            idxs_sb[:, :],
            128,
            128,
            nacc,
            elem_step=RES_STRIDE,
            prepare_only=True,
            sem=fs,
        ).then_inc(ps, 1)
        # ds last: ps/zs resolve long before the final DVE op, so the SEQ
        # sits parked on ds and the trigger dispatches right after it fires
        nc.gpsimd.wait_ge(ps, 1)
        nc.gpsimd.wait_ge(zs, 16)
        nc.gpsimd.wait_ge(ds, 1)
        nc.gpsimd.trigger_dma(1)
        nc.gpsimd.wait_ge(fs, 16)

    if cfg.get("hoist"):
        blk = nc.m.functions[0].blocks[0]
        insts = blk.instructions
        for eng in (mybir.EngineType.SP, mybir.EngineType.Activation):
            first_pre = None
            dma_idx = None
            for i, inst in enumerate(insts):
                if inst.engine == eng:
                    if first_pre is None:
                        first_pre = i
                    if type(inst).__name__ == "InstDMACopy":
                        dma_idx = i
                        break
            if dma_idx is not None and first_pre is not None and dma_idx > first_pre:
                dma = insts[dma_idx]
                del insts[dma_idx]
                insts.insert(first_pre, dma)
    nc.compile()
    nc._ant_plan = (chunks, ops, tot, ilv)  # host-side gather map
    return nc


def _pack31(bits):
    lead = bits.shape[:-1]
    if PAYLOAD == 32:
        words = np.packbits(bits, axis=-1, bitorder="little")
        return words.view(np.uint32).view(np.float32)
    b32 = np.zeros(lead + (WPK, 32), dtype=np.uint8)
    pad = np.zeros(lead + (DPAD,), dtype=np.uint8)
    pad[..., :D] = bits
    pad = pad.reshape(lead + (WPK, PAYLOAD))
    b32[..., :30] = pad[..., :30]
    b32[..., 31] = pad[..., 30]
    words = np.packbits(b32.reshape(lead + (WPK * 32,)), axis=-1, bitorder="little")
    return words.view(np.uint32).view(np.float32)


def _pack_inputs(x, bit_weights, tot, ilv=False):
    x = np.asarray(x).astype(np.uint8)
    bw = np.ascontiguousarray(np.asarray(bit_weights).astype(np.uint8))
    notx = (1 - x).astype(np.uint8)
    nxp = _pack31(notx)  # [WPK]
    wp = _pack31(bw)  # [L, WPK]
    in_maps = []
    for i in range(NCORES):
        shard = wp[i * LSH : (i + 1) * LSH].reshape(128, NB, WPK)
        wxa = np.zeros((128, tot), dtype=np.float32)
        if ilv:
            wxa[:, 0 : 2 * WPK : 2] = nxp
            wxa[:, 1 : 2 * WPK : 2] = shard[:, 0, :]
            wxa[:, 2 * WPK : TOT_W] = shard[:, 1:, :].reshape(128, ROWS_W - WPK)
        else:
            wxa[:, 0:WPK] = nxp
            wxa[:, WPK:TOT_W] = shard.reshape(128, ROWS_W)
        in_maps.append({"wx": wxa})
    return in_maps


def _gather(results, ops):
    outs = []
    for i in range(NCORES):
        res = results[i]["res"]  # [128, RES_STRIDE] fp32 flags
        viol = np.zeros((128, NB), dtype=bool)
        for j, (r, _, _, _) in enumerate(ops):
            viol[:, r] |= res[:, j] != 0.0
        outs.append(~viol.reshape(-1))
    return np.concatenate(outs).astype(np.bool_)


def _get_compiled():
    global _compiled
    if _compiled is None:
        _compiled = _build()
    return _compiled


def kernel(x, bit_weights):
    from concourse import bass_utils

    nc = _get_compiled()
    chunks, ops, tot, ilv = nc._ant_plan
    in_maps = _pack_inputs(x, bit_weights, tot, ilv)
    r = bass_utils.run_bass_kernel_spmd(nc, in_maps, core_ids=list(range(NCORES)))
    return _gather(r.results, ops)


# revision 7
# speedup vs baseline: 1.0717x; 1.0027x over previous
"""HardAndLayer on 8 Trainium2 NeuronCores — raw-bass v2.

out[l] = AND_d (x[d] OR NOT w[l,d])  ==  no d with (w[l,d] AND NOT x[d])

Wire format: 31-bit packed words (bit 30 held zero, so no fp32 NaN/Inf
pattern). Hand-rolled bass program (no TileContext): manual semaphores,
HWDGE input chunks sized so the DVE starts early and the DMA bus never
starves, and the output leaves via a pre-staged SWDGE scatter-add fired by
trigger_dma right after the last DVE op (skips HWDGE 625ns + DGE 650ns on
the tail). res DRAM is pre-zeroed by a small DMA; the LOGICAL_OR fold
yields exact 0.0/1.0 flags, so the fp32 scatter-ADD is bit-exact.
Host: out[neuron] = all per-piece flags == 0.
"""

import numpy as np

L = 8192
D = 8192
NCORES = 8
LSH = L // NCORES  # 1024 neuron rows per core
# 31 bits per word with bit 30 held zero: no word can form a NaN/Inf
# pattern. Dense 32-bit packing was tried and FAILS on HW — an all-ones
# notx word (x=0) is a NaN pattern and the DVE read datapath mangles it
# (1780/8192 mismatches on the adversarial x=all-zeros case).
PAYLOAD = 31
WPK = -(-D // PAYLOAD)  # packed words per neuron row
NB = LSH // 128  # 8 rows per partition

ROWS_W = NB * WPK  # 2120 row words per partition
TOT_W = (NB + 1) * WPK  # + notx
RES_STRIDE = 64  # res row stride in f32 (256B scatter-add constraint)

# ---- schedule config (tuned against TimelineSim) ----
CFG = {
    # interleaved layout [ilv(nx,r0) | r1..r7]: chunk 0 carries notx+r0 as
    # pairs, so the DVE starts ~140ns earlier; rows 1-7 read notx through a
    # stride-2 AP. Bounds are words over that layout (9*WPK total).
    "interleave": True,
    "bounds": (0, 702, 1108, 1540, 1887, 2218, 2385),
    # alternate SP/Act DMA issue: two sequencers feed the global HWDGE unit,
    # so descriptor-gen slots pace at 625ns instead of one engine's 650ns
    "alt_engines": True,
    # number of leading chunks delivered via Pool SWDGE prepare+trigger
    # (gather with identity idxs) instead of HWDGE dma_start. A trig chunk's
    # width must be a multiple of 64 words (256B gather elem constraint).
    "n_trig": 0,
    "zero_mode": "pool",  # "sp_last" | "pool" (pool keeps zs off the trigger path)
    "min_piece": 8,  # merge op pieces smaller than this into neighbor
    # "scatter": pre-staged SWDGE scatter-add fired by trigger_dma (fast tail)
    # "plain": SP HWDGE dma_start of acc -> res after DVE done (safe fallback)
    "out_mode": "scatter",
    # hoist the first SP/Act input DMAs ahead of the framework entry barrier
    # in their engine streams: the first transfer has no cross-engine
    # dependency, so it can overlap the ~616ns preamble (first byte ~1300
    # instead of ~1916)
    "hoist": True,
}

_BITPOS = list(range(30)) + [31]
DPAD = WPK * PAYLOAD

_compiled = None
_custom_op = None


def _register_custom_op():
    global _custom_op
    if _custom_op is not None:
        return _custom_op
    from concourse import dve_ops
    from concourse.dve_spec import Spec, Src0, Src1, Zero, Bin, lower
    from concourse.dve_uop import AluOp, DveOpSpec

    name = "AND_ANY_ANT"
    for o in dve_ops.OPS:
        if o.name == name:
            _custom_op = o
            return o

    def _ref(in0, in1, c0, c1, c2):
        a = in0.view(np.uint32) & in1.view(np.uint32)
        acc = (
            (a.reshape(a.shape[0], -1) != 0)
            .any(axis=-1, keepdims=True)
            .astype(np.float32)
        )
        return a.view(np.float32), acc

    spec = Spec(
        body=Bin(AluOp.BITWISE_AND, Src0, Src1),
        accum=AluOp.LOGICAL_OR,
        accum_init=Zero,
        reference=_ref,
    )
    shas = {}
    for ver in ("v3", "v4"):
        try:
            uops = lower(spec, ver=ver)
            shas[ver] = DveOpSpec(name=name, uops=uops, rd1_en=True).sha(ver)
        except Exception:
            pass
    op = dve_ops.DveOp(name, spec, subdim=False, uops_sha=shas)
    dve_ops.OPS.append(op)
    dve_ops._SUB_OPCODE_FOR_NAME[name] = (
        dve_ops._CUSTOM_DVE_ROW_BASE + len(dve_ops.OPS) - 1
    )
    dve_ops.CUSTOM_DVE_SPECS[name] = spec
    _custom_op = op
    return op


def _plan_ilv(cfg):
    """Interleaved layout: [ilv(nx,r0) 2*WPK | r1 .. r7]. Chunk 0 can be
    half-sized (notx+r0 words arrive as pairs), starting the DVE earlier;
    rows 1-7 read notx through a stride-2 AP over the interleave region.

    ops: (row, w0, w1, wait_chunks) with row-relative word ranges.
    """
    bounds = list(cfg["bounds"])
    assert bounds[0] == 0 and bounds[-1] == TOT_W
    ILV = 2 * WPK
    for b in bounds:
        if b < ILV:
            assert b % 2 == 0, "bounds inside the interleave region must be even"
    chunks = [(bounds[i], bounds[i + 1]) for i in range(len(bounds) - 1)]

    def chunk_of(col):
        for ci, (a, b) in enumerate(chunks):
            if a <= col < b:
                return ci
        raise AssertionError(col)

    ilv_chunk = chunk_of(ILV - 1)  # last chunk holding interleave words
    min_piece = cfg.get("min_piece", 8)

    def pieces_of(lo, hi, cutpts):
        cuts = [lo] + [c for c in cutpts if lo < c < hi] + [hi]
        out = []
        for i in range(len(cuts) - 1):
            a, b = cuts[i], cuts[i + 1]
            if out and (b - a) < min_piece:
                pa, _ = out.pop()
                out.append((pa, b))
            else:
                out.append((a, b))
        return out

    ops = []
    # r0: pair-space cuts at even bounds inside [0, ILV)
    cutp = [b // 2 for b in bounds if 0 < b < ILV]
    for a, b in pieces_of(0, WPK, cutp):
        ops.append((0, a, b, (chunk_of(2 * b - 1),)))
    # rows 1-7
    for r in range(1, NB):
        base = ILV + (r - 1) * WPK
        cutw = [b - base for b in bounds if base < b < base + WPK]
        for a, b in pieces_of(0, WPK, cutw):
            ops.append((r, a, b, (chunk_of(base + b - 1), ilv_chunk)))
    return chunks, ops


def _plan(cfg):
    """Derive (chunks, ops) from cfg.

    chunks: list of (w0, w1) word ranges over [notx | rows] layout.
    ops: list of (row, w0, w1, chunk_idx) — row-relative word range, the op
    is issued after `chunk_idx`'s DMA lands (notx chunk 0 is waited first).
    """
    bounds = list(cfg["bounds"])
    assert bounds[0] == 0 and bounds[-1] == TOT_W
    assert bounds[1] >= WPK, "chunk 0 must cover notx"
    chunks = [(bounds[i], bounds[i + 1]) for i in range(len(bounds) - 1)]

    def chunk_of(col):
        for ci, (a, b) in enumerate(chunks):
            if a <= col < b:
                return ci
        raise AssertionError(col)

    min_piece = cfg.get("min_piece", 8)
    ops = []
    for r in range(NB):
        lo, hi = (1 + r) * WPK, (2 + r) * WPK
        cuts = [lo] + [b for b in bounds if lo < b < hi] + [hi]
        # merge pieces smaller than min_piece into the previous piece
        pieces = []
        for i in range(len(cuts) - 1):
            a, b = cuts[i], cuts[i + 1]
            if pieces and (b - a) < min_piece:
                pa, _ = pieces.pop()
                pieces.append((pa, b))
            else:
                pieces.append((a, b))
        for a, b in pieces:
            ops.append((r, a - lo, b - lo, chunk_of(b - 1)))
    return chunks, ops


def _build(cfg=None):
    import concourse.bacc as bacc
    import concourse.mybir as mybir
    from concourse.library_config import mlp
    from contextlib import ExitStack

    if cfg is None:
        cfg = CFG
    op = _register_custom_op()
    ilv = bool(cfg.get("interleave"))
    chunks, ops = (_plan_ilv if ilv else _plan)(cfg)
    n_trig = cfg.get("n_trig", 0)
    for a, b in chunks[:n_trig]:
        assert (b - a) % 64 == 0, "trig chunk width must be 256B-aligned"
    nacc = len(ops)
    assert nacc <= RES_STRIDE
    zcol = TOT_W  # zero-source words live after the weights
    tot = -(-(TOT_W + nacc) // 64) * 64  # row stride %64 (gather elem_step)

    nc = bacc.Bacc(
        "TRN2",
        target_bir_lowering=False,
        debug=False,
        enable_asserts=False,
        num_devices=NCORES,
    )
    wx = nc.dram_tensor("wx", [128, tot], mybir.dt.float32, kind="ExternalInput")
    res = nc.dram_tensor(
        "res", [128, RES_STRIDE], mybir.dt.float32, kind="ExternalOutput"
    )
    # NOTE: no other ExternalInput may be declared — bass2jax passes exactly
    # the tensors in in_maps; a declared-but-unfed input fails the execute.

    with (
        ExitStack() as stack,
        nc.sbuf_tensor("wsb", [128, TOT_W], mybir.dt.float32) as wsb,
        nc.sbuf_tensor("idxs_sb", [128, 8], mybir.dt.int16) as idxs_sb,
        nc.sbuf_tensor("acc", [128, 1, RES_STRIDE], mybir.dt.float32) as acc,
        nc.sbuf_tensor("m", [128, len(ops), WPK], mybir.dt.float32) as m,
        nc.semaphore("zs") as zs,
        nc.semaphore("ds") as ds,
        nc.semaphore("ps") as ps,
        nc.semaphore("fs") as fs,
    ):
        csems = [
            stack.enter_context(nc.semaphore(f"c{i}")) for i in range(len(chunks))
        ]
        gps = stack.enter_context(nc.semaphore("gps"))

        # --- SP/Act: input chunks via HWDGE, then res pre-zero. Two
        # sequencers feed the (global) HWDGE unit so descriptor-gen slots
        # pace at 625ns instead of one engine's 650ns DMA_SEQ_TIME. Note
        # DGE_DMA_DELAY differs per engine (SP 650 vs Act 784), so the
        # per-DMA engine assignment is schedule-searched, not just
        # alternated. "engines": 0=sync, 1=scalar per DMA (zero DMA last).
        alt = cfg.get("alt_engines", False)
        engs = cfg.get("engines")
        if engs is None:
            n_dma = len(chunks) - n_trig + 1
            engs = tuple(i % 2 if alt else 0 for i in range(n_dma))
        hw_i = 0
        for ci, (a, b) in enumerate(chunks):
            if ci < n_trig:
                continue
            eng = nc.scalar if engs[hw_i] else nc.sync
            eng.dma_start(wsb[:, a:b], wx[:, a:b]).then_inc(csems[ci], 16)
            hw_i += 1
        if cfg["zero_mode"] == "sp_last" and cfg.get("out_mode", "scatter") == "scatter":
            eng = nc.scalar if engs[hw_i] else nc.sync
            eng.dma_start(res[:, 0:nacc], wx[:, zcol : zcol + nacc]).then_inc(
                zs, 16
            )

        # --- DVE: fused AND+any per op piece ---
        waited = set()
        nx_ap = wsb[:, 0:WPK]
        inst = None
        for j, (r, w0, w1, ci) in enumerate(ops):
            need = ci if isinstance(ci, tuple) else (0, ci)
            for c in sorted(set(need)):
                if c not in waited:
                    nc.vector.wait_ge(csems[c], 16)
                    waited.add(c)
            if ilv:
                if r == 0:
                    in0 = wsb[:, 2 * w0 + 1 : 2 * w1 : 2]
                    in1 = wsb[:, 2 * w0 : 2 * w1 : 2]
                else:
                    base = (1 + r) * WPK
                    in0 = wsb[:, base + w0 : base + w1]
                    in1 = wsb[:, 2 * w0 : 2 * w1 : 2]
            else:
                in0 = wsb[:, (1 + r) * WPK + w0 : (1 + r) * WPK + w1]
                in1 = nx_ap[:, w0:w1]
            inst = nc.vector._custom_dve(
                op,
                out=m[:, j, 0 : w1 - w0],
                in0=in0,
                in1=in1,
                accum_out=acc[:, 0, j : j + 1],
            )
        inst.then_inc(ds, 1)

        if cfg.get("out_mode", "scatter") == "plain":
            # safe fallback: plain HWDGE out-DMA on SP after DVE done
            nc.sync.wait_ge(ds, 1)
            nc.sync.dma_start(res[:, 0:nacc], acc[:, 0, 0:nacc]).then_inc(fs, 16)
            nc.sync.wait_ge(fs, 16)
            nc.compile()
            nc._ant_plan = (chunks, ops, tot, ilv)
            return nc

        # --- Pool: identity idxs via iota (boot lib = standard), early input
        # chunks + scatter prep staged in the SWDGE ring; input triggers fire
        # immediately, the scatter trigger right after the last DVE op ---
        isem = stack.enter_context(nc.semaphore("isem"))
        nc.gpsimd.memset(idxs_sb[:, :], 0).then_inc(isem, 1)
        nc.gpsimd.wait_ge(isem, 1)
        nc.gpsimd.iota(
            idxs_sb[0:16, :], [[16, 8]], base=0, channel_multiplier=1
        ).then_inc(isem, 1)
        nc.gpsimd.load_library(mlp)
        nc.gpsimd.wait_ge(isem, 2)
        for ci in range(n_trig):
            a, b = chunks[ci]
            nc.gpsimd.dma_gather(
                wsb[:, a:b].unsqueeze(1),
                wx[:, a:b],
                idxs_sb[:, :],
                128,
                128,
                b - a,
                elem_step=tot,
                prepare_only=True,
                sem=csems[ci],
            ).then_inc(gps, 1)
            nc.gpsimd.wait_ge(gps, ci + 1)
            nc.gpsimd.trigger_dma(1)
        if cfg["zero_mode"] == "pool":
            nc.gpsimd.dma_start(
                res[:, 0:nacc], wx[:, zcol : zcol + nacc]
            ).then_inc(zs, 16)
        nc.gpsimd.dma_scatter_add(
            res[:, 0:nacc],
            acc[:, :, 0:nacc],
            idxs_sb[:, :],
            128,
            128,
            nacc,
            elem_step=RES_STRIDE,
            prepare_only=True,
            sem=fs,
        ).then_inc(ps, 1)
        # ds last: ps/zs resolve long before the final DVE op, so the SEQ
        # sits parked on ds and the trigger dispatches right after it fires
        nc.gpsimd.wait_ge(ps, 1)
        nc.gpsimd.wait_ge(zs, 16)
        nc.gpsimd.wait_ge(ds, 1)
        nc.gpsimd.trigger_dma(1)
        nc.gpsimd.wait_ge(fs, 16)

    if cfg.get("hoist"):
        # move each engine's first input DMACopy — and the res pre-zero DMA
        # (last SP DMACopy) — ahead of that engine's preamble barrier; they
        # have no cross-engine dependencies, so they overlap the ~616ns
        # entry barrier and the zero sem clears the trigger path early
        blk = nc.m.functions[0].blocks[0]
        insts = blk.instructions
        for eng in (mybir.EngineType.SP, mybir.EngineType.Activation):
            first_pre = None
            idxs = []
            for i, inst in enumerate(insts):
                if inst.engine == eng:
                    if first_pre is None:
                        first_pre = i
                    if type(inst).__name__ == "InstDMACopy":
                        idxs.append(i)
            moves = [idxs[0]] if idxs else []
            if eng == mybir.EngineType.SP and len(idxs) > 1 and cfg.get(
                "hoist_zero"
            ) and cfg.get(
                "zero_mode"
            ) == "sp_last" and cfg.get("out_mode", "scatter") == "scatter":
                moves.append(idxs[-1])  # the res pre-zero DMA
            for k, di in enumerate(sorted(moves)):
                if first_pre is None or di <= first_pre:
                    continue
                dma = insts[di]
                del insts[di]
                insts.insert(first_pre + k, dma)
    nc.compile()
    nc._ant_plan = (chunks, ops, tot, ilv)  # host-side gather map
    return nc


def _pack31(bits):
    lead = bits.shape[:-1]
    if PAYLOAD == 32:
        words = np.packbits(bits, axis=-1, bitorder="little")
        return words.view(np.uint32).view(np.float32)
    b32 = np.zeros(lead + (WPK, 32), dtype=np.uint8)
    pad = np.zeros(lead + (DPAD,), dtype=np.uint8)
    pad[..., :D] = bits
    pad = pad.reshape(lead + (WPK, PAYLOAD))
    b32[..., :30] = pad[..., :30]
    b32[..., 31] = pad[..., 30]
    words = np.packbits(b32.reshape(lead + (WPK * 32,)), axis=-1, bitorder="little")
    return words.view(np.uint32).view(np.float32)


def _pack_inputs(x, bit_weights, tot, ilv=False):
    x = np.asarray(x).astype(np.uint8)
    bw = np.ascontiguousarray(np.asarray(bit_weights).astype(np.uint8))
    notx = (1 - x).astype(np.uint8)
    nxp = _pack31(notx)  # [WPK]
    wp = _pack31(bw)  # [L, WPK]
    in_maps = []
    for i in range(NCORES):
        shard = wp[i * LSH : (i + 1) * LSH].reshape(128, NB, WPK)
        wxa = np.zeros((128, tot), dtype=np.float32)
        if ilv:
            wxa[:, 0 : 2 * WPK : 2] = nxp
            wxa[:, 1 : 2 * WPK : 2] = shard[:, 0, :]
            wxa[:, 2 * WPK : TOT_W] = shard[:, 1:, :].reshape(128, ROWS_W - WPK)
        else:
            wxa[:, 0:WPK] = nxp
            wxa[:, WPK:TOT_W] = shard.reshape(128, ROWS_W)
        in_maps.append({"wx": wxa})
    return in_maps


def _gather(results, ops):
    outs = []
    for i in range(NCORES):
        res = results[i]["res"]  # [128, RES_STRIDE] fp32 flags
        viol = np.zeros((128, NB), dtype=bool)
        for j, (r, _, _, _) in enumerate(ops):
            viol[:, r] |= res[:, j] != 0.0
        outs.append(~viol.reshape(-1))
    return np.concatenate(outs).astype(np.bool_)


def _get_compiled():
    global _compiled
    if _compiled is None:
        _compiled = _build()
    return _compiled


def kernel(x, bit_weights):
    from concourse import bass_utils

    nc = _get_compiled()
    chunks, ops, tot, ilv = nc._ant_plan
    in_maps = _pack_inputs(x, bit_weights, tot, ilv)
    r = bass_utils.run_bass_kernel_spmd(nc, in_maps, core_ids=list(range(NCORES)))
    return _gather(r.results, ops)


# revision 8
# speedup vs baseline: 1.0729x; 1.0011x over previous
"""HardAndLayer on 8 Trainium2 NeuronCores — raw-bass v2.

out[l] = AND_d (x[d] OR NOT w[l,d])  ==  no d with (w[l,d] AND NOT x[d])

Wire format: 31-bit packed words (bit 30 held zero, so no fp32 NaN/Inf
pattern). Hand-rolled bass program (no TileContext): manual semaphores,
HWDGE input chunks sized so the DVE starts early and the DMA bus never
starves, and the output leaves via a pre-staged SWDGE scatter-add fired by
trigger_dma right after the last DVE op (skips HWDGE 625ns + DGE 650ns on
the tail). res DRAM is pre-zeroed by a small DMA; the LOGICAL_OR fold
yields exact 0.0/1.0 flags, so the fp32 scatter-ADD is bit-exact.
Host: out[neuron] = all per-piece flags == 0.
"""

import numpy as np

L = 8192
D = 8192
NCORES = 8
LSH = L // NCORES  # 1024 neuron rows per core
# 31 bits per word with bit 30 held zero: no word can form a NaN/Inf
# pattern. Dense 32-bit packing was tried and FAILS on HW — an all-ones
# notx word (x=0) is a NaN pattern and the DVE read datapath mangles it
# (1780/8192 mismatches on the adversarial x=all-zeros case).
PAYLOAD = 31
WPK = -(-D // PAYLOAD)  # packed words per neuron row
NB = LSH // 128  # 8 rows per partition

ROWS_W = NB * WPK  # 2120 row words per partition
TOT_W = (NB + 1) * WPK  # + notx
RES_STRIDE = 64  # res row stride in f32 (256B scatter-add constraint)

# ---- schedule config (tuned against TimelineSim) ----
CFG = {
    # interleaved layout [ilv(nx,r0) | r1..r7]: chunk 0 carries notx+r0 as
    # pairs, so the DVE starts ~140ns earlier; rows 1-7 read notx through a
    # stride-2 AP. Bounds are words over that layout (9*WPK total).
    "interleave": True,
    "bounds": (0, 702, 1108, 1540, 1887, 2218, 2385),
    # alternate SP/Act DMA issue: two sequencers feed the global HWDGE unit,
    # so descriptor-gen slots pace at 625ns instead of one engine's 650ns
    "alt_engines": True,
    # number of leading chunks delivered via Pool SWDGE prepare+trigger
    # (gather with identity idxs) instead of HWDGE dma_start. A trig chunk's
    # width must be a multiple of 64 words (256B gather elem constraint).
    "n_trig": 0,
    "zero_mode": "pool",  # "sp_last" | "pool" (pool keeps zs off the trigger path)
    "min_piece": 8,  # merge op pieces smaller than this into neighbor
    # "scatter": pre-staged SWDGE scatter-add fired by trigger_dma (fast tail)
    # "plain": SP HWDGE dma_start of acc -> res after DVE done (safe fallback)
    "out_mode": "scatter",
    # hoist the first SP/Act input DMAs ahead of the framework entry barrier
    # in their engine streams: the first transfer has no cross-engine
    # dependency, so it can overlap the ~616ns preamble (first byte ~1300
    # instead of ~1916)
    "hoist": True,
}

_BITPOS = list(range(30)) + [31]
DPAD = WPK * PAYLOAD

_compiled = None
_custom_op = None


def _register_custom_op():
    global _custom_op
    if _custom_op is not None:
        return _custom_op
    from concourse import dve_ops
    from concourse.dve_spec import Spec, Src0, Src1, Zero, Bin, lower
    from concourse.dve_uop import AluOp, DveOpSpec

    name = "AND_ANY_ANT"
    for o in dve_ops.OPS:
        if o.name == name:
            _custom_op = o
            return o

    def _ref(in0, in1, c0, c1, c2):
        a = in0.view(np.uint32) & in1.view(np.uint32)
        acc = (
            (a.reshape(a.shape[0], -1) != 0)
            .any(axis=-1, keepdims=True)
            .astype(np.float32)
        )
        return a.view(np.float32), acc

    spec = Spec(
        body=Bin(AluOp.BITWISE_AND, Src0, Src1),
        accum=AluOp.LOGICAL_OR,
        accum_init=Zero,
        reference=_ref,
    )
    shas = {}
    for ver in ("v3", "v4"):
        try:
            uops = lower(spec, ver=ver)
            shas[ver] = DveOpSpec(name=name, uops=uops, rd1_en=True).sha(ver)
        except Exception:
            pass
    op = dve_ops.DveOp(name, spec, subdim=False, uops_sha=shas)
    dve_ops.OPS.append(op)
    dve_ops._SUB_OPCODE_FOR_NAME[name] = (
        dve_ops._CUSTOM_DVE_ROW_BASE + len(dve_ops.OPS) - 1
    )
    dve_ops.CUSTOM_DVE_SPECS[name] = spec
    _custom_op = op
    return op


def _plan_ilv(cfg):
    """Interleaved layout: [ilv(nx,r0) 2*WPK | r1 .. r7]. Chunk 0 can be
    half-sized (notx+r0 words arrive as pairs), starting the DVE earlier;
    rows 1-7 read notx through a stride-2 AP over the interleave region.

    ops: (row, w0, w1, wait_chunks) with row-relative word ranges.
    """
    bounds = list(cfg["bounds"])
    assert bounds[0] == 0 and bounds[-1] == TOT_W
    ILV = 2 * WPK
    for b in bounds:
        if b < ILV:
            assert b % 2 == 0, "bounds inside the interleave region must be even"
    chunks = [(bounds[i], bounds[i + 1]) for i in range(len(bounds) - 1)]

    def chunk_of(col):
        for ci, (a, b) in enumerate(chunks):
            if a <= col < b:
                return ci
        raise AssertionError(col)

    ilv_chunk = chunk_of(ILV - 1)  # last chunk holding interleave words
    min_piece = cfg.get("min_piece", 8)

    def pieces_of(lo, hi, cutpts):
        cuts = [lo] + [c for c in cutpts if lo < c < hi] + [hi]
        out = []
        for i in range(len(cuts) - 1):
            a, b = cuts[i], cuts[i + 1]
            if out and (b - a) < min_piece:
                pa, _ = out.pop()
                out.append((pa, b))
            else:
                out.append((a, b))
        return out

    ops = []
    # r0: pair-space cuts at even bounds inside [0, ILV)
    cutp = [b // 2 for b in bounds if 0 < b < ILV]
    for a, b in pieces_of(0, WPK, cutp):
        ops.append((0, a, b, (chunk_of(2 * b - 1),)))
    # rows 1-7
    for r in range(1, NB):
        base = ILV + (r - 1) * WPK
        cutw = [b - base for b in bounds if base < b < base + WPK]
        for a, b in pieces_of(0, WPK, cutw):
            ops.append((r, a, b, (chunk_of(base + b - 1), ilv_chunk)))
    return chunks, ops


def _plan(cfg):
    """Derive (chunks, ops) from cfg.

    chunks: list of (w0, w1) word ranges over [notx | rows] layout.
    ops: list of (row, w0, w1, chunk_idx) — row-relative word range, the op
    is issued after `chunk_idx`'s DMA lands (notx chunk 0 is waited first).
    """
    bounds = list(cfg["bounds"])
    assert bounds[0] == 0 and bounds[-1] == TOT_W
    assert bounds[1] >= WPK, "chunk 0 must cover notx"
    chunks = [(bounds[i], bounds[i + 1]) for i in range(len(bounds) - 1)]

    def chunk_of(col):
        for ci, (a, b) in enumerate(chunks):
            if a <= col < b:
                return ci
        raise AssertionError(col)

    min_piece = cfg.get("min_piece", 8)
    ops = []
    for r in range(NB):
        lo, hi = (1 + r) * WPK, (2 + r) * WPK
        cuts = [lo] + [b for b in bounds if lo < b < hi] + [hi]
        # merge pieces smaller than min_piece into the previous piece
        pieces = []
        for i in range(len(cuts) - 1):
            a, b = cuts[i], cuts[i + 1]
            if pieces and (b - a) < min_piece:
                pa, _ = pieces.pop()
                pieces.append((pa, b))
            else:
                pieces.append((a, b))
        for a, b in pieces:
            ops.append((r, a - lo, b - lo, chunk_of(b - 1)))
    return chunks, ops


def _build(cfg=None):
    import concourse.bacc as bacc
    import concourse.mybir as mybir
    from concourse.library_config import mlp
    from contextlib import ExitStack

    if cfg is None:
        cfg = CFG
    op = _register_custom_op()
    ilv = bool(cfg.get("interleave"))
    chunks, ops = (_plan_ilv if ilv else _plan)(cfg)
    n_trig = cfg.get("n_trig", 0)
    for a, b in chunks[:n_trig]:
        assert (b - a) % 64 == 0, "trig chunk width must be 256B-aligned"
    nacc = len(ops)
    assert nacc <= RES_STRIDE
    zcol = TOT_W  # zero-source words live after the weights
    tot = -(-(TOT_W + nacc) // 64) * 64  # row stride %64 (gather elem_step)

    nc = bacc.Bacc(
        "TRN2",
        target_bir_lowering=False,
        debug=False,
        enable_asserts=False,
        num_devices=NCORES,
    )
    wx = nc.dram_tensor("wx", [128, tot], mybir.dt.float32, kind="ExternalInput")
    res = nc.dram_tensor(
        "res", [128, RES_STRIDE], mybir.dt.float32, kind="ExternalOutput"
    )
    # NOTE: no other ExternalInput may be declared — bass2jax passes exactly
    # the tensors in in_maps; a declared-but-unfed input fails the execute.

    with (
        ExitStack() as stack,
        nc.sbuf_tensor("wsb", [128, TOT_W], mybir.dt.float32) as wsb,
        nc.sbuf_tensor("idxs_sb", [128, 8], mybir.dt.int16) as idxs_sb,
        nc.sbuf_tensor("acc", [128, 1, RES_STRIDE], mybir.dt.float32) as acc,
        nc.sbuf_tensor("m", [128, len(ops), WPK], mybir.dt.float32) as m,
        nc.semaphore("zs") as zs,
        nc.semaphore("ds") as ds,
        nc.semaphore("ps") as ps,
        nc.semaphore("fs") as fs,
    ):
        csems = [
            stack.enter_context(nc.semaphore(f"c{i}")) for i in range(len(chunks))
        ]
        gps = stack.enter_context(nc.semaphore("gps"))

        # --- SP/Act: input chunks via HWDGE, then res pre-zero. Two
        # sequencers feed the (global) HWDGE unit so descriptor-gen slots
        # pace at 625ns instead of one engine's 650ns DMA_SEQ_TIME. Note
        # DGE_DMA_DELAY differs per engine (SP 650 vs Act 784), so the
        # per-DMA engine assignment is schedule-searched, not just
        # alternated. "engines": 0=sync, 1=scalar per DMA (zero DMA last).
        alt = cfg.get("alt_engines", False)
        engs = cfg.get("engines")
        if engs is None:
            n_dma = len(chunks) - n_trig + 1
            engs = tuple(i % 2 if alt else 0 for i in range(n_dma))
        hw_i = 0
        for ci, (a, b) in enumerate(chunks):
            if ci < n_trig:
                continue
            eng = nc.scalar if engs[hw_i] else nc.sync
            eng.dma_start(wsb[:, a:b], wx[:, a:b]).then_inc(csems[ci], 16)
            hw_i += 1
        if cfg["zero_mode"] == "sp_last" and cfg.get("out_mode", "scatter") == "scatter":
            eng = nc.scalar if engs[hw_i] else nc.sync
            eng.dma_start(res[:, 0:nacc], wx[:, zcol : zcol + nacc]).then_inc(
                zs, 16
            )

        # --- DVE: fused AND+any per op piece ---
        waited = set()
        nx_ap = wsb[:, 0:WPK]
        inst = None
        for j, (r, w0, w1, ci) in enumerate(ops):
            need = ci if isinstance(ci, tuple) else (0, ci)
            for c in sorted(set(need)):
                if c not in waited:
                    nc.vector.wait_ge(csems[c], 16)
                    waited.add(c)
            if ilv:
                if r == 0:
                    in0 = wsb[:, 2 * w0 + 1 : 2 * w1 : 2]
                    in1 = wsb[:, 2 * w0 : 2 * w1 : 2]
                else:
                    base = (1 + r) * WPK
                    in0 = wsb[:, base + w0 : base + w1]
                    in1 = wsb[:, 2 * w0 : 2 * w1 : 2]
            else:
                in0 = wsb[:, (1 + r) * WPK + w0 : (1 + r) * WPK + w1]
                in1 = nx_ap[:, w0:w1]
            inst = nc.vector._custom_dve(
                op,
                out=m[:, j, 0 : w1 - w0],
                in0=in0,
                in1=in1,
                accum_out=acc[:, 0, j : j + 1],
            )
        inst.then_inc(ds, 1)

        if cfg.get("out_mode", "scatter") == "plain":
            # safe fallback: plain HWDGE out-DMA on SP after DVE done
            nc.sync.wait_ge(ds, 1)
            nc.sync.dma_start(res[:, 0:nacc], acc[:, 0, 0:nacc]).then_inc(fs, 16)
            nc.sync.wait_ge(fs, 16)
            nc.compile()
            nc._ant_plan = (chunks, ops, tot, ilv)
            return nc

        # --- Pool: identity idxs via iota (boot lib = standard), early input
        # chunks + scatter prep staged in the SWDGE ring; input triggers fire
        # immediately, the scatter trigger right after the last DVE op ---
        isem = stack.enter_context(nc.semaphore("isem"))
        nc.gpsimd.memset(idxs_sb[:, :], 0).then_inc(isem, 1)
        nc.gpsimd.wait_ge(isem, 1)
        nc.gpsimd.iota(
            idxs_sb[0:16, :], [[16, 8]], base=0, channel_multiplier=1
        ).then_inc(isem, 1)
        nc.gpsimd.load_library(mlp)
        nc.gpsimd.wait_ge(isem, 2)
        for ci in range(n_trig):
            a, b = chunks[ci]
            nc.gpsimd.dma_gather(
                wsb[:, a:b].unsqueeze(1),
                wx[:, a:b],
                idxs_sb[:, :],
                128,
                128,
                b - a,
                elem_step=tot,
                prepare_only=True,
                sem=csems[ci],
            ).then_inc(gps, 1)
            nc.gpsimd.wait_ge(gps, ci + 1)
            nc.gpsimd.trigger_dma(1)
        if cfg["zero_mode"] == "pool":
            nc.gpsimd.dma_start(
                res[:, 0:nacc], wx[:, zcol : zcol + nacc]
            ).then_inc(zs, 16)
        nc.gpsimd.dma_scatter_add(
            res[:, 0:nacc],
            acc[:, :, 0:nacc],
            idxs_sb[:, :],
            128,
            128,
            nacc,
            elem_step=RES_STRIDE,
            prepare_only=True,
            sem=fs,
        ).then_inc(ps, 1)
        # ds last: ps/zs resolve long before the final DVE op, so the SEQ
        # sits parked on ds and the trigger dispatches right after it fires
        nc.gpsimd.wait_ge(ps, 1)
        nc.gpsimd.wait_ge(zs, 16)
        nc.gpsimd.wait_ge(ds, 1)
        nc.gpsimd.trigger_dma(1)
        # terminal wait on SP: its sem receive overhead is 0 (vs Pool's 8)
        # and SEQ overhead 25 (vs 36), so the program end lands earlier
        nc.sync.wait_ge(fs, 16)

    if cfg.get("hoist"):
        # move each engine's first input DMACopy — and the res pre-zero DMA
        # (last SP DMACopy) — ahead of that engine's preamble barrier; they
        # have no cross-engine dependencies, so they overlap the ~616ns
        # entry barrier and the zero sem clears the trigger path early
        blk = nc.m.functions[0].blocks[0]
        insts = blk.instructions
        for eng in (mybir.EngineType.SP, mybir.EngineType.Activation):
            first_pre = None
            idxs = []
            for i, inst in enumerate(insts):
                if inst.engine == eng:
                    if first_pre is None:
                        first_pre = i
                    if type(inst).__name__ == "InstDMACopy":
                        idxs.append(i)
            moves = [idxs[0]] if idxs else []
            if eng == mybir.EngineType.SP and len(idxs) > 1 and cfg.get(
                "hoist_zero"
            ) and cfg.get(
                "zero_mode"
            ) == "sp_last" and cfg.get("out_mode", "scatter") == "scatter":
                moves.append(idxs[-1])  # the res pre-zero DMA
            for k, di in enumerate(sorted(moves)):
                if first_pre is None or di <= first_pre:
                    continue
                dma = insts[di]
                del insts[di]
                insts.insert(first_pre + k, dma)
    nc.compile()
    nc._ant_plan = (chunks, ops, tot, ilv)  # host-side gather map
    return nc


def _pack31(bits):
    lead = bits.shape[:-1]
    if PAYLOAD == 32:
        words = np.packbits(bits, axis=-1, bitorder="little")
        return words.view(np.uint32).view(np.float32)
    b32 = np.zeros(lead + (WPK, 32), dtype=np.uint8)
    pad = np.zeros(lead + (DPAD,), dtype=np.uint8)
    pad[..., :D] = bits
    pad = pad.reshape(lead + (WPK, PAYLOAD))
    b32[..., :30] = pad[..., :30]
    b32[..., 31] = pad[..., 30]
    words = np.packbits(b32.reshape(lead + (WPK * 32,)), axis=-1, bitorder="little")
    return words.view(np.uint32).view(np.float32)


def _pack_inputs(x, bit_weights, tot, ilv=False):
    x = np.asarray(x).astype(np.uint8)
    bw = np.ascontiguousarray(np.asarray(bit_weights).astype(np.uint8))
    notx = (1 - x).astype(np.uint8)
    nxp = _pack31(notx)  # [WPK]
    wp = _pack31(bw)  # [L, WPK]
    in_maps = []
    for i in range(NCORES):
        shard = wp[i * LSH : (i + 1) * LSH].reshape(128, NB, WPK)
        wxa = np.zeros((128, tot), dtype=np.float32)
        if ilv:
            wxa[:, 0 : 2 * WPK : 2] = nxp
            wxa[:, 1 : 2 * WPK : 2] = shard[:, 0, :]
            wxa[:, 2 * WPK : TOT_W] = shard[:, 1:, :].reshape(128, ROWS_W - WPK)
        else:
            wxa[:, 0:WPK] = nxp
            wxa[:, WPK:TOT_W] = shard.reshape(128, ROWS_W)
        in_maps.append({"wx": wxa})
    return in_maps


def _gather(results, ops):
    outs = []
    for i in range(NCORES):
        res = results[i]["res"]  # [128, RES_STRIDE] fp32 flags
        viol = np.zeros((128, NB), dtype=bool)
        for j, (r, _, _, _) in enumerate(ops):
            viol[:, r] |= res[:, j] != 0.0
        outs.append(~viol.reshape(-1))
    return np.concatenate(outs).astype(np.bool_)


def _get_compiled():
    global _compiled
    if _compiled is None:
        _compiled = _build()
    return _compiled


def kernel(x, bit_weights):
    from concourse import bass_utils

    nc = _get_compiled()
    chunks, ops, tot, ilv = nc._ant_plan
    in_maps = _pack_inputs(x, bit_weights, tot, ilv)
    r = bass_utils.run_bass_kernel_spmd(nc, in_maps, core_ids=list(range(NCORES)))
    return _gather(r.results, ops)
